# revision 37
# baseline (speedup 1.0000x reference)
"""CNF forward (vector field + exact Jacobian trace) on 8 TRN2 cores.

Math: reference computes, per sample x (row of state[:, 1:]):
    f(x)  = W3^T tanh(W2^T tanh(W1^T [x; t] + b1) + b2) + b3      (dx)
    trJ   = trace(df/dx)                                          (aug = -trJ)

Closed form of the trace (instead of D=64 JVPs per sample):
    h1 = tanh([x;t] @ W1 + b1),  h2 = tanh(h1 @ W2 + b2)
    s1 = 1 - h1^2,  q2 = h2^2 - 1
    aug = -trJ = sum_h (s1^T F)[b,h] * q2[b,h]
    with F[h',h] = W2[h',h] * (W3 @ W1[:D])[h, h'] (weights-only, on device)

Sharding: data-parallel, 128 samples per core, weights replicated.

Measured constraints that shape this kernel (perfetto/ntff evidence):
- The PE is effectively stuck cold at 1.2 GHz (HAM rarely releases).
- Each DMA-issuing engine (scalar/sync/gpsimd) owns ONE queue;
  in-order per queue, ~180-250 GB/s aggregate, first data ~8.4us
  (engine wake + DIRECT2D descriptor generation ~0.65us each).
  TOTAL BYTES set when the last weight lands, so every tensor is
  minimal and 128-partition (64/65-partition descriptors cost 1-1.7us
  of DIRECT2D vs ~0.6us).

Design:
- FEATURE-MAJOR everywhere ([h, b] tiles): h2 lands feature-major so
  dx = h2 @ W3 needs no PE transposes / staging copies; the aug
  contraction over h runs on the PE against a ones[128,1] stationary.
- z1 and G share one flat W1x stationary (both K=64, base 0); being
  unpacked costs ~1us of PE in windows where the PE idles on DMA
  anyway, and avoids duplicated stT/W3^T halves (-81KB of weights:
  total bytes set when the last w2 tile's semaphore fires, which gates
  z2). b1 + t*W1[D] rides as 4 columns of the w3cb tensor (loaded
  first on its queue) and is applied by the ACT bias in the z1 tanh.
- Each G bank is retired by a cheap scalar PSUM->SBUF copy (not the
  w2-gated F multiply), so the G stream never stalls on the ring, and
  the F = W2 (*) G multiplies split across vector AND gpsimd.
- z2 is k-OUTER in w2-tile DMA arrival order, with its LAST k-group
  emitted after 9 prefill t2 matmuls (j0-j2 x k0-k2, operands already
  resident, banks borrowed from retired G/dx slots) that hide the
  last w2 tile's DMA-semaphore wait. t2 is j-OUTER so each t2_j
  retires early and the aug pipeline drains behind it.
- s1 is split vector/gpsimd (gpsimd alone serializes 2.7us and gated
  t2 in a previous iteration).
- PSUM: one 4-bank [128,128]f32 ring serves z1 -> z2 -> t2; 2 banks
  for G pairs, 1 for dx, 1 for aug = exactly 8. No two engines ever
  touch one PSUM bank concurrently (hardware collisions are fatal).
- aug is reduced batch-major ([BC,1], w_fm stationary x ones moving)
  so it merges into ONE [BC, D+1] fp16 output DMA on the sync queue
  (host upcasts): a second DMA costs ~1-2us of cold-queue wake latency
  at the end (measured both for a tiny aug DMA and for row-half
  splitting across sync+gpsimd — gpsimd's descriptor wakes ~0.75us
  late and its half landed later than the single transfer).

All matmul operands fp16, accumulation fp32 PSUM; l2 rel err ~5e-4
(gate 2e-2). Host-side work is layout/cast only (sharding, transposes,
casts, bias/ones packing, row duplication); all FLOPs run on device.
"""

import numpy as np

import concourse.bacc as bacc
import concourse.bass as bass
import concourse.tile as tile
from concourse import mybir
from concourse.bass_utils import run_bass_kernel_spmd
from concourse.tile_rust import add_dep_helper

B, D, H = 1024, 64, 512
NCORES = 8
BC = B // NCORES  # 128 samples per core
KT = H // 128     # 4 feature tiles of 128
F32 = mybir.dt.float32
F16 = mybir.dt.float16
AF = mybir.ActivationFunctionType
ALU = mybir.AluOpType
ts = bass.ts

_NC = {}

Z2_KORD = [0, 1, 2, 3]  # w2 k-tile DMA arrival order

# One queue per issuing engine; in-order per queue. First use first.
DMA_PLAN = [
    ("scalar", "stT"), ("sync", "w1xf"), ("gpsimd", "w3cb"),
    ("scalar", "w2_0"), ("sync", "w2_1"), ("gpsimd", "w3T"),
    ("scalar", "w2_3"), ("gpsimd", "w2_2"),
]


def _build(with_bias23: bool):
    """with_bias23: include b2/b3 adds (b2 via ACT bias columns, b3 via
    a ones-row rank-1 matmul). setup_inputs() has zero biases so the
    fast path skips them; nonzero still works."""
    nc = bacc.Bacc()

    # x^T shard (z1 moving operand), plain 64-partition
    STT = nc.declare_dram_parameter("stT", [D, BC], F16, isOutput=False)
    # W1x flat: shared stationary for z1 AND G (both K=64, base 0)
    W1XF = nc.declare_dram_parameter("w1xf", [D, H], F16, isOutput=False)
    W2 = nc.declare_dram_parameter("W2", [H, H], F16, isOutput=False)
    # w3cb: cols 0-3 = b1_eff columns (ACT bias), then W3 packed
    # [128, KT*64] (block k = W3[k*128:(k+1)*128, :]). Loaded FIRST on
    # its queue so the z1 tanh bias is resident early.
    W3CB = nc.declare_dram_parameter("w3cb", [128, KT + KT * D], F16,
                                     isOutput=False)
    # W3^T plain (G moving operand)
    W3T = nc.declare_dram_parameter("w3T", [D, H], F16, isOutput=False)
    if with_bias23:
        b2c = nc.declare_dram_parameter("b2c", [128, KT], F16, isOutput=False)
        b3r = nc.declare_dram_parameter("b3r", [1, D], F16, isOutput=False)
    # fp16 output (host upcasts): halves copy+DMA bytes; upstream math
    # is fp16-limited so this adds <0.05% to the error budget.
    out = nc.declare_dram_parameter("out", [BC, D + 1], F16, isOutput=True)
    # scratch sink for the sync-queue warmer (never read back)
    qwarm = nc.dram_tensor("qwarm", [1, D], F16)

    with tile.TileContext(nc) as tc:
        with (
            tc.tile_pool(name="const", bufs=1) as cp,
            tc.tile_pool(name="act", bufs=1) as ap,
            tc.tile_pool(name="ps", bufs=1, space="PSUM") as ps,
        ):
            # ------------- loads (plan set by DMA_PLAN) -------------
            stT_sb = ap.tile([D, BC], F16, tag="stT")
            w1xf = cp.tile([D, H], F16, tag="w1xf")
            w2_sb = [cp.tile([128, H], F16, tag=f"w2_{k}", name=f"w2_{k}")
                     for k in range(KT)]
            w3T_sb = cp.tile([D, H], F16, tag="w3T")
            w3cb = cp.tile([128, KT + KT * D], F16, tag="w3cb")
            w3cat = w3cb[:, KT:KT + KT * D]
            srcs = {"stT": (stT_sb, STT), "w1xf": (w1xf, W1XF),
                    "w3T": (w3T_sb, W3T), "w3cb": (w3cb, W3CB)}
            for k in range(KT):
                srcs[f"w2_{k}"] = (w2_sb[k], W2[ts(k, 128), :])
            for eng, nm in DMA_PLAN:
                dst, src = srcs[nm]
                src = src if isinstance(src, bass.AP) else src[:, :]
                dst = dst if isinstance(dst, bass.AP) else dst[:, :]
                getattr(nc, eng).dma_start(out=dst, in_=src)
            if with_bias23:
                b2c_sb = cp.tile([128, KT], F16, tag="b2c")
                nc.sync.dma_start(out=b2c_sb, in_=b2c[:, :])
                b3r_sb = cp.tile([1, D], F16, tag="b3r")
                nc.sync.dma_start(out=b3r_sb, in_=b3r[:, :])
                onesr = cp.tile([1, BC], F16, tag="onesr")
                nc.vector.memset(onesr, 1.0)
            # ones column: stationary vector for the aug reduction
            onesc = cp.tile([128, 1], F16, tag="onesc")
            nc.vector.memset(onesc, 1.0)

            # ------------- layer 1 (row-packed K=64 pairs) ----------
            h1 = [None] * KT
            z1_ps = [None] * KT
            z1_mm = [None] * KT
            for j in range(KT):
                z1_ps[j] = ps.tile([128, BC], F32, tag="fm", bufs=4,
                                   name=f"z1_{j}")
                z1_mm[j] = nc.tensor.matmul(
                    z1_ps[j], w1xf[:, ts(j, 128)], stT_sb,
                    start=True, stop=True)
            for j in range(KT):
                h = ap.tile([128, BC], F16, tag=f"h1_{j}")
                nc.scalar.activation(h, z1_ps[j], AF.Tanh,
                                     bias=w3cb[:, j:j + 1])
                h1[j] = h

            # s1 = 1 - h1^2, split vector/gpsimd (feature-major, fp16)
            s1 = []
            for j in range(KT):
                s = ap.tile([128, BC], F16, tag=f"s1_{j}")
                eng = nc.vector if j < 2 else nc.gpsimd
                eng.tensor_mul(s, h1[j], h1[j])
                eng.tensor_scalar(s, s, -1.0, 1.0, ALU.mult, ALU.add)
                s1.append(s)

            # ------------- G (trace weights) ------------------------
            # shares the z1 stationary (W1x flat, K=64). A cheap scalar
            # PSUM->SBUF copy retires each G bank immediately (the bank
            # reader is no longer the w2-gated F multiply), and the
            # F = W2 (*) G multiplies split across vector AND gpsimd.
            f_sb, g_mm = [None] * KT, [None] * KT

            def emit_g(m):
                g_ps = ps.tile([128, H], F32, tag="g", bufs=3)
                g_mm[m] = nc.tensor.matmul(
                    g_ps, w1xf[:, ts(m, 128)], w3T_sb,
                    start=True, stop=True)
                gs = cp.tile([128, H], F16, tag=f"gs_{m}")
                nc.scalar.copy(gs, g_ps)
                fm = ap.tile([128, H], F16, tag=f"f_{m}")
                eng = nc.vector if m % 2 == 0 else nc.gpsimd
                eng.tensor_mul(fm, w2_sb[m], gs)
                f_sb[m] = fm

            for m in range(KT):
                emit_g(m)
            add_dep_helper(g_mm[0].ins, z1_mm[KT - 1].ins, sync=False,
                           reason="pe G after z1")

            # ------------- layer 2 (feature-major, k-outer) ---------
            # the last k-group is emitted separately: 9 t2 prefill
            # matmuls (j0-j2 x k0-k2, operands already resident) fill
            # the PE while it waits for the last w2 tile's DMA sem.
            z2_ps = [ps.tile([128, BC], F32, tag="fm", bufs=4,
                             name=f"z2_{j}") for j in range(KT)]
            z2_mm = {}
            klast = Z2_KORD[-1]
            for ki, k in enumerate(Z2_KORD[:-1]):
                for j in range(KT):
                    z2_mm[k, j] = nc.tensor.matmul(
                        z2_ps[j], w2_sb[k][:, ts(j, 128)], h1[k],
                        start=(ki == 0), stop=False)
            add_dep_helper(z2_mm[Z2_KORD[0], 0].ins, g_mm[KT - 1].ins,
                           sync=False, reason="pe z2 after G")

            # t2 prefill: j0-j2 accumulate k0-k2 into the retired G
            # banks (the fm ring still holds the open z2 tiles).
            # j2 borrows the dx bank: the next two g-ring slots would
            # be g3's bank, whose f3-multiply reader is itself gated on
            # the w2 tile this prefill is hiding the wait for.
            t2_ps = [None] * KT
            t2_mm = [[None] * KT for _ in range(KT)]  # [j][k]
            for j in range(KT - 1):
                if j < 2:
                    t2_ps[j] = ps.tile([128, BC], F32, tag="g", bufs=3,
                                       name=f"t2_{j}")
                else:
                    t2_ps[j] = ps.tile([128, BC], F32, tag="o", bufs=1,
                                       name=f"t2_{j}")
                for k in range(KT - 1):
                    t2_mm[j][k] = nc.tensor.matmul(
                        t2_ps[j], f_sb[k][:, ts(j, 128)], s1[k],
                        start=(k == 0), stop=False)
            add_dep_helper(t2_mm[0][0].ins, z2_mm[Z2_KORD[-2], KT - 1].ins,
                           sync=False, reason="pe t2 prefill in w2 wait")

            # last z2 k-group, then the t2 k-last completions
            for j in range(KT):
                z2_mm[klast, j] = nc.tensor.matmul(
                    z2_ps[j], w2_sb[klast][:, ts(j, 128)], h1[klast],
                    start=False, stop=True)
            add_dep_helper(z2_mm[klast, 0].ins,
                           t2_mm[KT - 2][KT - 2].ins,
                           sync=False, reason="pe z2 klast after prefill")
            for j in range(KT - 1):
                t2_mm[j][KT - 1] = nc.tensor.matmul(
                    t2_ps[j], f_sb[KT - 1][:, ts(j, 128)], s1[KT - 1],
                    start=False, stop=True)
            add_dep_helper(t2_mm[0][KT - 1].ins, z2_mm[klast, KT - 1].ins,
                           sync=False, reason="pe t2 klast after z2")

            h2, q2 = [None] * KT, [None] * KT
            for j in range(KT):
                h = ap.tile([128, BC], F16, tag=f"h2_{j}")
                if with_bias23:
                    nc.scalar.activation(h, z2_ps[j], AF.Tanh,
                                         bias=b2c_sb[:, j:j + 1])
                else:
                    nc.scalar.activation(h, z2_ps[j], AF.Tanh)
                h2[j] = h
                # q = h2^2 - 1 (aug = sum t2*q needs no extra terms)
                q = ap.tile([128, BC], F16, tag=f"q2_{j}")
                eng = nc.gpsimd if j % 2 else nc.vector
                eng.tensor_mul(q, h, h)
                eng.tensor_scalar(q, q, 1.0, -1.0, ALU.mult, ALU.add)
                q2[j] = q
            # sync-queue warmer: a tiny mid-kernel DMA (gated on h2_0,
            # ~5us before the output) keeps the sync DMA queue from
            # going cold — a cold queue adds ~0.8-1us doorbell-to-data
            # latency to the final output transfer.
            nc.sync.dma_start(out=qwarm[:, :], in_=h2[0][0:1, 0:D])

            # t2_j3 (fm ring slot, free once tanh_0 retires z2_0)
            t2_ps[KT - 1] = ps.tile([128, BC], F32, tag="fm", bufs=4,
                                    name=f"t2_{KT - 1}")
            for k in range(KT):
                t2_mm[KT - 1][k] = nc.tensor.matmul(
                    t2_ps[KT - 1], f_sb[k][:, ts(KT - 1, 128)], s1[k],
                    start=(k == 0), stop=(k == KT - 1))
            add_dep_helper(t2_mm[KT - 1][0].ins, t2_mm[KT - 2][KT - 1].ins,
                           sync=False, reason="pe t2 j3 order")

            # ------------- layer 3: dx = h2 @ W3 (batch-major out) ---
            o_ps = ps.tile([BC, D], F32, tag="o", bufs=1)
            o_mm = [None] * KT
            for j in range(KT):
                o_mm[j] = nc.tensor.matmul(o_ps, h2[j], w3cat[:, ts(j, D)],
                                           start=(j == 0),
                                           stop=(j == KT - 1
                                                 and not with_bias23))
            if with_bias23:
                nc.tensor.matmul(o_ps, onesr, b3r_sb, start=False, stop=True)
            add_dep_helper(o_mm[0].ins, t2_mm[KT - 1][KT - 1].ins,
                           sync=False, reason="pe dx after t2")
            final_sb = ap.tile([BC, D + 1], F16, tag="final")
            nc.scalar.copy(final_sb[:, 1:D + 1], o_ps)

            # ------------- aug = sum_h t2*q2 via PE ones-reduction ---
            # w_fm is the STATIONARY side so aug lands batch-major
            # [BC, 1] and merges into the single [BC, D+1] output DMA
            # (a second tiny DMA costs ~2us of cold-queue latency).
            aug_ps = ps.tile([BC, 1], F32, tag="g", bufs=3)
            for j in range(KT):
                w = ap.tile([128, BC], F16, tag=f"wfm_{j}")
                nc.vector.tensor_mul(w, t2_ps[j], q2[j])
                mm = nc.tensor.matmul(aug_ps, w, onesc,
                                      start=(j == 0), stop=(j == KT - 1))
                if j == 0:
                    add_dep_helper(mm.ins, o_mm[KT - 1].ins,
                                   sync=False, reason="pe aug after dx")
            nc.vector.tensor_copy(final_sb[:, 0:1], aug_ps)
            nc.sync.dma_start(out=out[:, :], in_=final_sb)

    nc.finalize()
    return nc


def _get_nc(with_bias23: bool):
    key = bool(with_bias23)
    if key not in _NC:
        _NC[key] = _build(key)
    return _NC[key]


def make_in_maps(inputs):
    f32 = lambda a: np.ascontiguousarray(np.asarray(a), dtype=np.float32)
    f16 = lambda a: np.ascontiguousarray(np.asarray(a, dtype=np.float32),
                                         dtype=np.float16)
    state = f32(inputs["state"])
    t = float(np.asarray(inputs["t"]).reshape(-1)[0])
    W1 = f32(inputs["W1"])
    b1 = f32(inputs["b1"]).reshape(H)
    W2 = f16(inputs["W2"])
    b2 = f32(inputs["b2"]).reshape(H)
    W3 = f16(inputs["W3"])
    b3 = f32(inputs["b3"]).reshape(D)

    with_bias23 = bool(np.any(b2) or np.any(b3))

    b1_eff = b1 + t * W1[D]                  # fold t-row into bias cols
    W1x = W1[:D]

    b1c = np.zeros((128, KT), np.float32)
    for j in range(KT):
        b1c[:, j] = b1_eff[j * 128:(j + 1) * 128]
    W3c = np.concatenate(
        [b1c] + [W3[k * 128:(k + 1) * 128, :] for k in range(KT)], axis=1)
    base = {
        "w1xf": f16(W1x),
        "W2": W2,
        "w3cb": f16(W3c),
        "w3T": f16(W3.T),
    }
    if with_bias23:
        b2cols = np.zeros((128, KT), np.float32)
        for j in range(KT):
            b2cols[:, j] = b2[j * 128:(j + 1) * 128]
        base["b2c"] = f16(b2cols)
        base["b3r"] = f16(b3.reshape(1, D))
    in_maps = []
    for c in range(NCORES):
        m = dict(base)
        m["stT"] = f16(state[c * BC:(c + 1) * BC, 1:].T)
        in_maps.append(m)
    return with_bias23, in_maps


def kernel(**inputs) -> np.ndarray:
    with_bias23, in_maps = make_in_maps(inputs)
    res = run_bass_kernel_spmd(_get_nc(with_bias23), in_maps,
                               list(range(NCORES))).results
    return np.concatenate([res[c]["out"] for c in range(NCORES)],
                          axis=0).astype(np.float32)


# revision 38
# speedup vs baseline: 1.1596x; 1.1596x over previous
"""CNF forward (vector field + exact Jacobian trace) on 8 TRN2 cores.

Math: reference computes, per sample x (row of state[:, 1:]):
    f(x)  = W3^T tanh(W2^T tanh(W1^T [x; t] + b1) + b2) + b3      (dx)
    trJ   = trace(df/dx)                                          (aug = -trJ)

Closed form of the trace (instead of D=64 JVPs per sample):
    h1 = tanh([x;t] @ W1 + b1),  h2 = tanh(h1 @ W2 + b2)
    s1 = 1 - h1^2,  q2 = h2^2 - 1
    aug = -trJ = sum_h (s1^T F)[b,h] * q2[b,h]
    with F[h',h] = W2[h',h] * (W3 @ W1[:D])[h, h'] (weights-only, on device)

Sharding: data-parallel, 128 samples per core, weights replicated.

Measured constraints that shape this kernel (perfetto/ntff evidence):
- The PE is effectively stuck cold at 1.2 GHz (HAM rarely releases).
- Each DMA-issuing engine (scalar/sync/gpsimd) owns ONE queue;
  in-order per queue, ~180-250 GB/s aggregate, first data ~8.4us
  (engine wake + DIRECT2D descriptor generation ~0.65us each).
  TOTAL BYTES set when the last weight lands, so every tensor is
  minimal and 128-partition (64/65-partition descriptors cost 1-1.7us
  of DIRECT2D vs ~0.6us).

Design:
- FEATURE-MAJOR everywhere ([h, b] tiles): h2 lands feature-major so
  dx = h2 @ W3 needs no PE transposes / staging copies; the aug
  contraction over h runs on the PE against a ones[128,1] stationary.
- z1 and G share one flat W1x stationary (both K=64, base 0); being
  unpacked costs ~1us of PE in windows where the PE idles on DMA
  anyway, and avoids duplicated stT/W3^T halves (-81KB of weights:
  total bytes set when the last w2 tile's semaphore fires, which gates
  z2). b1 + t*W1[D] rides as 4 columns of the w3cb tensor (loaded
  first on its queue) and is applied by the ACT bias in the z1 tanh.
- Each G bank is retired by a cheap scalar PSUM->SBUF copy (not the
  w2-gated F multiply), so the G stream never stalls on the ring, and
  the F = W2 (*) G multiplies split across vector AND gpsimd.
- z2 is k-OUTER in w2-tile DMA arrival order, with its LAST k-group
  emitted after 9 prefill t2 matmuls (j0-j2 x k0-k2, operands already
  resident, banks borrowed from retired G/dx slots) that hide the
  last w2 tile's DMA-semaphore wait. t2 is j-OUTER so each t2_j
  retires early and the aug pipeline drains behind it.
- s1 is split vector/gpsimd (gpsimd alone serializes 2.7us and gated
  t2 in a previous iteration).
- PSUM: one 4-bank [128,128]f32 ring serves z1 -> z2 -> t2; 2 banks
  for G pairs, 1 for dx, 1 for aug = exactly 8. No two engines ever
  touch one PSUM bank concurrently (hardware collisions are fatal).
- aug is reduced batch-major ([BC,1], w_fm stationary x ones moving)
  so it merges into ONE [BC, D+1] fp16 output DMA on the sync queue
  (host upcasts): a second DMA costs ~1-2us of cold-queue wake latency
  at the end (measured both for a tiny aug DMA and for row-half
  splitting across sync+gpsimd — gpsimd's descriptor wakes ~0.75us
  late and its half landed later than the single transfer).

All matmul operands fp16, accumulation fp32 PSUM; l2 rel err ~5e-4
(gate 2e-2). Host-side work is layout/cast only (sharding, transposes,
casts, bias/ones packing, row duplication); all FLOPs run on device.
"""

import numpy as np

import concourse.bacc as bacc
import concourse.bass as bass
import concourse.tile as tile
from concourse import mybir
from concourse.bass_utils import run_bass_kernel_spmd
from concourse.tile_rust import add_dep_helper

B, D, H = 1024, 64, 512
NCORES = 8
BC = B // NCORES  # 128 samples per core
KT = H // 128     # 4 feature tiles of 128
F32 = mybir.dt.float32
F16 = mybir.dt.float16
AF = mybir.ActivationFunctionType
ALU = mybir.AluOpType
ts = bass.ts

_NC = {}

Z2_KORD = [0, 1, 2, 3]  # w2 k-tile DMA arrival order

# One queue per issuing engine; in-order per queue. First use first.
DMA_PLAN = [
    ("scalar", "stT"), ("sync", "w1xf"), ("gpsimd", "w3cb"),
    ("scalar", "w2_0"), ("sync", "w2_1"), ("gpsimd", "w3T"),
    ("scalar", "w2_3"), ("gpsimd", "w2_2"),
]


def _build(with_bias23: bool):
    """with_bias23: include b2/b3 adds (b2 via ACT bias columns, b3 via
    a ones-row rank-1 matmul). setup_inputs() has zero biases so the
    fast path skips them; nonzero still works."""
    nc = bacc.Bacc()

    # x^T shard (z1 moving operand), plain 64-partition
    STT = nc.declare_dram_parameter("stT", [D, BC], F16, isOutput=False)
    # W1x flat: shared stationary for z1 AND G (both K=64, base 0)
    W1XF = nc.declare_dram_parameter("w1xf", [D, H], F16, isOutput=False)
    W2 = nc.declare_dram_parameter("W2", [H, H], F16, isOutput=False)
    # w3cb: cols 0-3 = b1_eff columns (ACT bias), then W3 packed
    # [128, KT*64] (block k = W3[k*128:(k+1)*128, :]). Loaded FIRST on
    # its queue so the z1 tanh bias is resident early.
    W3CB = nc.declare_dram_parameter("w3cb", [128, KT + KT * D], F16,
                                     isOutput=False)
    # W3^T plain (G moving operand)
    W3T = nc.declare_dram_parameter("w3T", [D, H], F16, isOutput=False)
    if with_bias23:
        b2c = nc.declare_dram_parameter("b2c", [128, KT], F16, isOutput=False)
        b3r = nc.declare_dram_parameter("b3r", [1, D], F16, isOutput=False)
    # fp16 output (host upcasts): halves copy+DMA bytes; upstream math
    # is fp16-limited so this adds <0.05% to the error budget.
    out = nc.declare_dram_parameter("out", [BC, D + 1], F16, isOutput=True)

    with tile.TileContext(nc) as tc:
        with (
            tc.tile_pool(name="const", bufs=1) as cp,
            tc.tile_pool(name="act", bufs=1) as ap,
            tc.tile_pool(name="ps", bufs=1, space="PSUM") as ps,
        ):
            # ------------- loads (plan set by DMA_PLAN) -------------
            stT_sb = ap.tile([D, BC], F16, tag="stT")
            w1xf = cp.tile([D, H], F16, tag="w1xf")
            w2_sb = [cp.tile([128, H], F16, tag=f"w2_{k}", name=f"w2_{k}")
                     for k in range(KT)]
            w3T_sb = cp.tile([D, H], F16, tag="w3T")
            w3cb = cp.tile([128, KT + KT * D], F16, tag="w3cb")
            w3cat = w3cb[:, KT:KT + KT * D]
            srcs = {"stT": (stT_sb, STT), "w1xf": (w1xf, W1XF),
                    "w3T": (w3T_sb, W3T), "w3cb": (w3cb, W3CB)}
            for k in range(KT):
                srcs[f"w2_{k}"] = (w2_sb[k], W2[ts(k, 128), :])
            for eng, nm in DMA_PLAN:
                dst, src = srcs[nm]
                src = src if isinstance(src, bass.AP) else src[:, :]
                dst = dst if isinstance(dst, bass.AP) else dst[:, :]
                getattr(nc, eng).dma_start(out=dst, in_=src)
            if with_bias23:
                b2c_sb = cp.tile([128, KT], F16, tag="b2c")
                nc.sync.dma_start(out=b2c_sb, in_=b2c[:, :])
                b3r_sb = cp.tile([1, D], F16, tag="b3r")
                nc.sync.dma_start(out=b3r_sb, in_=b3r[:, :])
                onesr = cp.tile([1, BC], F16, tag="onesr")
                nc.vector.memset(onesr, 1.0)
            # ones column: stationary vector for the aug reduction
            onesc = cp.tile([128, 1], F16, tag="onesc")
            nc.vector.memset(onesc, 1.0)

            # ------------- layer 1 (row-packed K=64 pairs) ----------
            h1 = [None] * KT
            z1_ps = [None] * KT
            z1_mm = [None] * KT
            for j in range(KT):
                z1_ps[j] = ps.tile([128, BC], F32, tag="fm", bufs=4,
                                   name=f"z1_{j}")
                z1_mm[j] = nc.tensor.matmul(
                    z1_ps[j], w1xf[:, ts(j, 128)], stT_sb,
                    start=True, stop=True)
            for j in range(KT):
                h = ap.tile([128, BC], F16, tag=f"h1_{j}")
                nc.scalar.activation(h, z1_ps[j], AF.Tanh,
                                     bias=w3cb[:, j:j + 1])
                h1[j] = h

            # s1 = 1 - h1^2, split vector/gpsimd (feature-major, fp16)
            s1 = []
            for j in range(KT):
                s = ap.tile([128, BC], F16, tag=f"s1_{j}")
                eng = nc.vector if j < 2 else nc.gpsimd
                eng.tensor_mul(s, h1[j], h1[j])
                eng.tensor_scalar(s, s, -1.0, 1.0, ALU.mult, ALU.add)
                s1.append(s)

            # ------------- G (trace weights) ------------------------
            # shares the z1 stationary (W1x flat, K=64). A cheap scalar
            # PSUM->SBUF copy retires each G bank immediately (the bank
            # reader is no longer the w2-gated F multiply), and the
            # F = W2 (*) G multiplies split across vector AND gpsimd.
            f_sb, g_mm = [None] * KT, [None] * KT

            def emit_g(m):
                g_ps = ps.tile([128, H], F32, tag="g", bufs=3)
                g_mm[m] = nc.tensor.matmul(
                    g_ps, w1xf[:, ts(m, 128)], w3T_sb,
                    start=True, stop=True)
                gs = cp.tile([128, H], F16, tag=f"gs_{m}")
                nc.scalar.copy(gs, g_ps)
                fm = ap.tile([128, H], F16, tag=f"f_{m}")
                eng = nc.vector if m % 2 == 0 else nc.gpsimd
                eng.tensor_mul(fm, w2_sb[m], gs)
                f_sb[m] = fm

            for m in range(KT):
                emit_g(m)
            add_dep_helper(g_mm[0].ins, z1_mm[KT - 1].ins, sync=False,
                           reason="pe G after z1")

            # ------------- layer 2 (feature-major, k-outer) ---------
            # the last k-group is emitted separately: 9 t2 prefill
            # matmuls (j0-j2 x k0-k2, operands already resident) fill
            # the PE while it waits for the last w2 tile's DMA sem.
            z2_ps = [ps.tile([128, BC], F32, tag="fm", bufs=4,
                             name=f"z2_{j}") for j in range(KT)]
            z2_mm = {}
            klast = Z2_KORD[-1]
            for ki, k in enumerate(Z2_KORD[:-1]):
                for j in range(KT):
                    z2_mm[k, j] = nc.tensor.matmul(
                        z2_ps[j], w2_sb[k][:, ts(j, 128)], h1[k],
                        start=(ki == 0), stop=False)
            add_dep_helper(z2_mm[Z2_KORD[0], 0].ins, g_mm[KT - 1].ins,
                           sync=False, reason="pe z2 after G")

            # t2 prefill: j0-j2 accumulate k0-k2 into the retired G
            # banks (the fm ring still holds the open z2 tiles).
            # j2 borrows the dx bank: the next two g-ring slots would
            # be g3's bank, whose f3-multiply reader is itself gated on
            # the w2 tile this prefill is hiding the wait for.
            t2_ps = [None] * KT
            t2_mm = [[None] * KT for _ in range(KT)]  # [j][k]
            for j in range(KT - 1):
                if j < 2:
                    t2_ps[j] = ps.tile([128, BC], F32, tag="g", bufs=3,
                                       name=f"t2_{j}")
                else:
                    t2_ps[j] = ps.tile([128, BC], F32, tag="o", bufs=1,
                                       name=f"t2_{j}")
                for k in range(KT - 1):
                    t2_mm[j][k] = nc.tensor.matmul(
                        t2_ps[j], f_sb[k][:, ts(j, 128)], s1[k],
                        start=(k == 0), stop=False)
            add_dep_helper(t2_mm[0][0].ins, z2_mm[Z2_KORD[-2], KT - 1].ins,
                           sync=False, reason="pe t2 prefill in w2 wait")

            # last z2 k-group, then the t2 k-last completions
            for j in range(KT):
                z2_mm[klast, j] = nc.tensor.matmul(
                    z2_ps[j], w2_sb[klast][:, ts(j, 128)], h1[klast],
                    start=False, stop=True)
            add_dep_helper(z2_mm[klast, 0].ins,
                           t2_mm[KT - 2][KT - 2].ins,
                           sync=False, reason="pe z2 klast after prefill")
            for j in range(KT - 1):
                t2_mm[j][KT - 1] = nc.tensor.matmul(
                    t2_ps[j], f_sb[KT - 1][:, ts(j, 128)], s1[KT - 1],
                    start=False, stop=True)
            add_dep_helper(t2_mm[0][KT - 1].ins, z2_mm[klast, KT - 1].ins,
                           sync=False, reason="pe t2 klast after z2")

            h2, q2 = [None] * KT, [None] * KT
            for j in range(KT):
                h = ap.tile([128, BC], F16, tag=f"h2_{j}")
                if with_bias23:
                    nc.scalar.activation(h, z2_ps[j], AF.Tanh,
                                         bias=b2c_sb[:, j:j + 1])
                else:
                    nc.scalar.activation(h, z2_ps[j], AF.Tanh)
                h2[j] = h
                # q = h2^2 - 1 (aug = sum t2*q needs no extra terms)
                q = ap.tile([128, BC], F16, tag=f"q2_{j}")
                eng = nc.gpsimd if j % 2 else nc.vector
                eng.tensor_mul(q, h, h)
                eng.tensor_scalar(q, q, 1.0, -1.0, ALU.mult, ALU.add)
                q2[j] = q

            # t2_j3 (fm ring slot, free once tanh_0 retires z2_0)
            t2_ps[KT - 1] = ps.tile([128, BC], F32, tag="fm", bufs=4,
                                    name=f"t2_{KT - 1}")
            for k in range(KT):
                t2_mm[KT - 1][k] = nc.tensor.matmul(
                    t2_ps[KT - 1], f_sb[k][:, ts(KT - 1, 128)], s1[k],
                    start=(k == 0), stop=(k == KT - 1))
            add_dep_helper(t2_mm[KT - 1][0].ins, t2_mm[KT - 2][KT - 1].ins,
                           sync=False, reason="pe t2 j3 order")

            # ------------- layer 3: dx = h2 @ W3 (batch-major out) ---
            o_ps = ps.tile([BC, D], F32, tag="o", bufs=1)
            o_mm = [None] * KT
            for j in range(KT):
                o_mm[j] = nc.tensor.matmul(o_ps, h2[j], w3cat[:, ts(j, D)],
                                           start=(j == 0),
                                           stop=(j == KT - 1
                                                 and not with_bias23))
            if with_bias23:
                nc.tensor.matmul(o_ps, onesr, b3r_sb, start=False, stop=True)
            add_dep_helper(o_mm[0].ins, t2_mm[KT - 1][KT - 1].ins,
                           sync=False, reason="pe dx after t2")
            final_sb = ap.tile([BC, D + 1], F16, tag="final")
            nc.scalar.copy(final_sb[:, 1:D + 1], o_ps)

            # ------------- aug = sum_h t2*q2 via PE ones-reduction ---
            # w_fm is the STATIONARY side so aug lands batch-major
            # [BC, 1] and merges into the single [BC, D+1] output DMA
            # (a second tiny DMA costs ~2us of cold-queue latency).
            aug_ps = ps.tile([BC, 1], F32, tag="g", bufs=3)
            for j in range(KT):
                w = ap.tile([128, BC], F16, tag=f"wfm_{j}")
                nc.vector.tensor_mul(w, t2_ps[j], q2[j])
                mm = nc.tensor.matmul(aug_ps, w, onesc,
                                      start=(j == 0), stop=(j == KT - 1))
                if j == 0:
                    add_dep_helper(mm.ins, o_mm[KT - 1].ins,
                                   sync=False, reason="pe aug after dx")
            nc.vector.tensor_copy(final_sb[:, 0:1], aug_ps)
            nc.sync.dma_start(out=out[:, :], in_=final_sb)

    nc.finalize()
    return nc


def _get_nc(with_bias23: bool):
    key = bool(with_bias23)
    if key not in _NC:
        _NC[key] = _build(key)
    return _NC[key]


def make_in_maps(inputs):
    f32 = lambda a: np.ascontiguousarray(np.asarray(a), dtype=np.float32)
    f16 = lambda a: np.ascontiguousarray(np.asarray(a, dtype=np.float32),
                                         dtype=np.float16)
    state = f32(inputs["state"])
    t = float(np.asarray(inputs["t"]).reshape(-1)[0])
    W1 = f32(inputs["W1"])
    b1 = f32(inputs["b1"]).reshape(H)
    W2 = f16(inputs["W2"])
    b2 = f32(inputs["b2"]).reshape(H)
    W3 = f16(inputs["W3"])
    b3 = f32(inputs["b3"]).reshape(D)

    with_bias23 = bool(np.any(b2) or np.any(b3))

    b1_eff = b1 + t * W1[D]                  # fold t-row into bias cols
    W1x = W1[:D]

    b1c = np.zeros((128, KT), np.float32)
    for j in range(KT):
        b1c[:, j] = b1_eff[j * 128:(j + 1) * 128]
    W3c = np.concatenate(
        [b1c] + [W3[k * 128:(k + 1) * 128, :] for k in range(KT)], axis=1)
    base = {
        "w1xf": f16(W1x),
        "W2": W2,
        "w3cb": f16(W3c),
        "w3T": f16(W3.T),
    }
    if with_bias23:
        b2cols = np.zeros((128, KT), np.float32)
        for j in range(KT):
            b2cols[:, j] = b2[j * 128:(j + 1) * 128]
        base["b2c"] = f16(b2cols)
        base["b3r"] = f16(b3.reshape(1, D))
    in_maps = []
    for c in range(NCORES):
        m = dict(base)
        m["stT"] = f16(state[c * BC:(c + 1) * BC, 1:].T)
        in_maps.append(m)
    return with_bias23, in_maps


def kernel(**inputs) -> np.ndarray:
    with_bias23, in_maps = make_in_maps(inputs)
    res = run_bass_kernel_spmd(_get_nc(with_bias23), in_maps,
                               list(range(NCORES))).results
    return np.concatenate([res[c]["out"] for c in range(NCORES)],
                          axis=0).astype(np.float32)
